# revision 13
# baseline (speedup 1.0000x reference)
"""Trainium2 Bass kernel for the CoxPath GCN forward pass.

Computation (per batch element b):
    h1 = tanh(adj @ (x_b @ W1) + b1)         [P, H]
    h2 = tanh(adj @ (h1 @ W2) + b2)          [P, H]
    s  = tanh(h2 @ lw1 + lb1)                [P]
    out_b = concat(s, clinical_b) @ lw2 + lb2

Sharding: data-parallel over batch B across 8 cores (16 batch elems/core);
adj and all weights replicated. No collectives needed (forward only).

Device strategy (per core, per batch element):
  A: S1 = x_b @ W1          via lhsT = xT chunks (host pre-transposed), rhs = W1
  B: h1T = tanh((adj@S1).T) via lhsT = S1 chunks, rhs = adjT (host pre-transposed,
                            SBUF-resident across the whole kernel: 16 MB)
  C: S2 = h1 @ W2           via lhsT = h1T chunks, rhs = W2
  D: h2T = tanh((adj@S2).T) same as B
  E: s = tanh(lw1 . h2T)    M=1 matmuls, written into row b of a [16, P+C] z tile
  F: out = rowwise dot(z, lw2) + lb2 via one tensor_tensor_reduce at the end

All matmuls run in float32r (TF32-class, 1 cycle/row on the PE vs 4 for fp32).
"""

import os
import sys

for _p in ("/opt/trn_rl_repo", "/root/.axon_site/_ro/trn_rl_repo"):
    if os.path.isdir(_p) and _p not in sys.path:
        sys.path.insert(0, _p)

import numpy as np
from contextlib import ExitStack

import concourse.tile as tile
from concourse import bacc, mybir
from concourse import bass_utils

# Problem dims (hardcoded per contract)
B, PP, F, H, C = 128, 2048, 512, 256, 16
NCORES = 8
BPC = B // NCORES  # 16 batch elements per core

FP32 = mybir.dt.float32
FP32R = mybir.dt.float32r
TANH = mybir.ActivationFunctionType.Tanh
PART = 128  # SBUF partitions


def build_bass(bpc=BPC, pp=PP, f=F, h=H, c=C, nfree=512):
    """Build + compile the per-core Bass program. Returns the Bacc object."""
    KP = pp // PART      # p-dim 128-tiles (16)
    KF = f // PART       # f-dim chunks (4)
    MH = h // PART       # h-dim chunks (2)
    NB = pp // nfree     # 512-wide column blocks of the adj matmul (4)

    nc = bacc.Bacc("TRN2", target_bir_lowering=False, debug=False)

    xT = nc.dram_tensor("xT", (bpc, f, pp), FP32R, kind="ExternalInput").ap()
    adjT = nc.dram_tensor("adjT", (pp, pp), FP32R, kind="ExternalInput").ap()
    clin = nc.dram_tensor("clin", (bpc, c), FP32, kind="ExternalInput").ap()
    W1 = nc.dram_tensor("W1", (f, h), FP32R, kind="ExternalInput").ap()
    b1 = nc.dram_tensor("b1", (h,), FP32, kind="ExternalInput").ap()
    W2 = nc.dram_tensor("W2", (h, h), FP32R, kind="ExternalInput").ap()
    b2 = nc.dram_tensor("b2", (h,), FP32, kind="ExternalInput").ap()
    lw1 = nc.dram_tensor("lw1", (h,), FP32R, kind="ExternalInput").ap()
    lb1 = nc.dram_tensor("lb1", (1,), FP32, kind="ExternalInput").ap()
    lw2 = nc.dram_tensor("lw2", (pp + c,), FP32, kind="ExternalInput").ap()
    lb2 = nc.dram_tensor("lb2", (1,), FP32, kind="ExternalInput").ap()
    out = nc.dram_tensor("out", (bpc, 1), FP32, kind="ExternalOutput").ap()

    with tile.TileContext(nc) as tc:
        with ExitStack() as ctx:
            consts = ctx.enter_context(tc.tile_pool(name="consts", bufs=1))
            xt_pool = ctx.enter_context(tc.tile_pool(name="xt", bufs=12))
            s12_pool = ctx.enter_context(tc.tile_pool(name="s12", bufs=1))
            ht_pool = ctx.enter_context(tc.tile_pool(name="ht", bufs=1))
            ps_ac = ctx.enter_context(tc.tile_pool(name="ps_ac", bufs=2, space="PSUM"))
            ps_bd = ctx.enter_context(tc.tile_pool(name="ps_bd", bufs=4, space="PSUM"))
            ps_e = ctx.enter_context(tc.tile_pool(name="ps_e", bufs=2, space="PSUM"))

            # ---- constants / resident tensors ----
            w1_sb = consts.tile([PART, KF, h], FP32R, tag="w1", name="w1_sb")
            nc.sync.dma_start(w1_sb[:], W1.rearrange("(kc p) h -> p kc h", p=PART))
            w2_sb = consts.tile([PART, MH, h], FP32R, tag="w2", name="w2_sb")
            nc.sync.dma_start(w2_sb[:], W2.rearrange("(kc p) h -> p kc h", p=PART))

            b1_sb = consts.tile([PART, MH], FP32, tag="b1", name="b1_sb")
            nc.sync.dma_start(b1_sb[:], b1.rearrange("(kc p) -> p kc", p=PART))
            b2_sb = consts.tile([PART, MH], FP32, tag="b2", name="b2_sb")
            nc.sync.dma_start(b2_sb[:], b2.rearrange("(kc p) -> p kc", p=PART))
            lw1_sb = consts.tile([PART, MH], FP32R, tag="lw1", name="lw1_sb")
            nc.sync.dma_start(lw1_sb[:], lw1.rearrange("(kc p) -> p kc", p=PART))
            lb1_sb = consts.tile([1, 1], FP32, tag="lb1", name="lb1_sb")
            nc.sync.dma_start(lb1_sb[:], lb1[None, :])

            lw2row = consts.tile([1, pp], FP32, tag="lw2row", name="lw2row")
            nc.sync.dma_start(lw2row[:], lw2[None, 0:pp])
            lw2cb = consts.tile([bpc, c], FP32, tag="lw2cb", name="lw2cb")
            nc.sync.dma_start(lw2cb[:], lw2[None, pp:pp + c].to_broadcast((bpc, c)))
            lb2_sb = consts.tile([bpc, 1], FP32, tag="lb2", name="lb2_sb")
            nc.sync.dma_start(lb2_sb[:], lb2[None, :].to_broadcast((bpc, 1)))

            # base = clinical @ lw2[pp:] + lb2, written to out once; per-batch
            # s-dot is then DMA-accumulated into its row
            clin_sb = consts.tile([bpc, c], FP32, tag="clin", name="clin_sb")
            nc.sync.dma_start(clin_sb[:], clin[:])
            base_sb = consts.tile([bpc, 1], FP32, tag="base", name="base_sb")
            nc.vector.tensor_mul(out=clin_sb[:], in0=clin_sb[:], in1=lw2cb[:])
            nc.vector.reduce_sum(base_sb[:], clin_sb[:], axis=mybir.AxisListType.X)
            nc.vector.tensor_add(base_sb[:], base_sb[:], lb2_sb[:])
            nc.sync.dma_start(out[:], base_sb[:])

            # batch-0 xT prefetch goes out BEFORE the 16 MB adjT load so the
            # PE can start phase A at t~2us instead of queueing behind it
            xt0_tiles = []
            xTb0 = xT[0].rearrange("(kc p) q -> p kc q", p=PART)
            for m in range(KP):
                xt0 = xt_pool.tile([PART, KF, PART], FP32R, tag="xt",
                                   name=f"xt0_{m}")
                nc.sync.dma_start(xt0[:], xTb0[:, :, m * PART:(m + 1) * PART])
                xt0_tiles.append(xt0)

            adjt_sb = []
            for k in range(KP):
                t = consts.tile([PART, pp], FP32R, tag=f"adjt_{k}", name=f"adjt_{k}")
                nc.sync.dma_start(t[:], adjT[k * PART:(k + 1) * PART, :])
                adjt_sb.append(t)

            # ---- per-batch pipeline ----
            for b in range(bpc):
                xTb = xT[b].rearrange("(kc p) q -> p kc q", p=PART)

                # Phase A: S1 = x_b @ W1  -> KP tiles [128, h] (fp32r)
                s1_tiles = []
                for m in range(KP):
                    if b == 0:
                        xt = xt0_tiles[m]
                    else:
                        xt = xt_pool.tile([PART, KF, PART], FP32R, tag="xt",
                                          name=f"xt_{b}_{m}")
                        nc.sync.dma_start(xt[:], xTb[:, :, m * PART:(m + 1) * PART])
                    ps = ps_ac.tile([PART, h], FP32, tag="ac", name=f"psa_{b}_{m}")
                    for kc in range(KF):
                        nc.tensor.matmul(ps[:], xt[:, kc, :], w1_sb[:, kc, :],
                                         start=(kc == 0), stop=(kc == KF - 1))
                    s1m = s12_pool.tile([PART, h], FP32R, tag=f"s12_{m}",
                                        name=f"s1_{b}_{m}")
                    nc.vector.tensor_copy(s1m[:], ps[:])
                    s1_tiles.append(s1m)

                # Phase B: h1T = tanh((adj @ S1).T + b1) -> MH tiles [128, pp]
                h1t = [ht_pool.tile([PART, pp], FP32R, tag=f"ht_{mh}",
                                    name=f"h1t_{b}_{mh}") for mh in range(MH)]
                if b == 0 and MH * NB <= 8:
                    # batch 0 runs while adjT is still streaming in: put all
                    # MH*NB accumulations in flight (borrowing psum slots from
                    # every pool) so each matmul only needs ITS k-tile of adjT
                    # and the PE fills the 16 MB load window instead of
                    # stalling on the last tile of the first chunk.
                    ps0 = []
                    pools = [ps_bd] * NB + [ps_ac, ps_ac, ps_e, ps_e][:max(0, MH * NB - NB)]
                    for i in range(MH * NB):
                        pool_i = pools[i] if i < len(pools) else ps_bd
                        ps0.append(pool_i.tile([PART, nfree], FP32,
                                               tag=["bd", "ac", "e"][0 if pool_i is ps_bd else (1 if pool_i is ps_ac else 2)],
                                               name=f"psb0_{i}"))
                    for k in range(KP):
                        for i in range(MH * NB):
                            mh, n = divmod(i, NB)
                            nc.tensor.matmul(
                                ps0[i][:],
                                s1_tiles[k][:, mh * PART:(mh + 1) * PART],
                                adjt_sb[k][:, n * nfree:(n + 1) * nfree],
                                start=(k == 0), stop=(k == KP - 1))
                    for i in range(MH * NB):
                        mh, n = divmod(i, NB)
                        nc.scalar.activation(
                            h1t[mh][:, n * nfree:(n + 1) * nfree], ps0[i][:],
                            TANH, bias=b1_sb[:, mh:mh + 1])
                else:
                    for mh in range(MH):
                        for n in range(NB):
                            ps = ps_bd.tile([PART, nfree], FP32, tag="bd",
                                            name=f"psb_{b}_{mh}_{n}")
                            for k in range(KP):
                                nc.tensor.matmul(
                                    ps[:],
                                    s1_tiles[k][:, mh * PART:(mh + 1) * PART],
                                    adjt_sb[k][:, n * nfree:(n + 1) * nfree],
                                    start=(k == 0), stop=(k == KP - 1))
                            nc.scalar.activation(h1t[mh][:, n * nfree:(n + 1) * nfree],
                                                 ps[:], TANH, bias=b1_sb[:, mh:mh + 1])

                # Phase C: S2 = h1 @ W2 -> KP tiles [128, h] (reuses s12 slots)
                s2_tiles = []
                for m in range(KP):
                    ps = ps_ac.tile([PART, h], FP32, tag="ac", name=f"psc_{b}_{m}")
                    for kc in range(MH):
                        nc.tensor.matmul(ps[:],
                                         h1t[kc][:, m * PART:(m + 1) * PART],
                                         w2_sb[:, kc, :],
                                         start=(kc == 0), stop=(kc == MH - 1))
                    s2m = s12_pool.tile([PART, h], FP32R, tag=f"s12_{m}",
                                        name=f"s2_{b}_{m}")
                    nc.vector.tensor_copy(s2m[:], ps[:])
                    s2_tiles.append(s2m)

                # Phase D: h2T = tanh((adj @ S2).T + b2) -> MH tiles [128, pp]
                h2t = []
                for mh in range(MH):
                    hm = ht_pool.tile([PART, pp], FP32R, tag=f"ht_{mh}",
                                      name=f"h2t_{b}_{mh}")
                    for n in range(NB):
                        ps = ps_bd.tile([PART, nfree], FP32, tag="bd",
                                        name=f"psd_{b}_{mh}_{n}")
                        for k in range(KP):
                            nc.tensor.matmul(
                                ps[:],
                                s2_tiles[k][:, mh * PART:(mh + 1) * PART],
                                adjt_sb[k][:, n * nfree:(n + 1) * nfree],
                                start=(k == 0), stop=(k == KP - 1))
                        nc.scalar.activation(hm[:, n * nfree:(n + 1) * nfree], ps[:],
                                             TANH, bias=b2_sb[:, mh:mh + 1])
                    h2t.append(hm)

                # Phase E: s = tanh(lw1 . h2T + lb1) -> row b of zall
                # (compute engines may only address partition starts 0/32/64/96,
                #  so tanh lands in a partition-0 row tile, DMA'd into row b)
                zrow = xt_pool.tile([1, pp], FP32, tag="zrow", name=f"zrow_{b}",
                                    bufs=1)
                for n in range(NB):
                    ps = ps_e.tile([1, nfree], FP32, tag="e", name=f"pse_{b}_{n}")
                    for kc in range(MH):
                        nc.tensor.matmul(ps[:],
                                         lw1_sb[:, kc:kc + 1],
                                         h2t[kc][:, n * nfree:(n + 1) * nfree],
                                         start=(kc == 0), stop=(kc == MH - 1))
                    nc.scalar.activation(zrow[:, n * nfree:(n + 1) * nfree],
                                         ps[:], TANH, bias=lb1_sb[:, :])
                nc.vector.tensor_mul(out=zrow[:], in0=zrow[:], in1=lw2row[:])
                spart = xt_pool.tile([1, 1], FP32, tag="spart", name=f"sp_{b}",
                                     bufs=2)
                nc.vector.reduce_sum(spart[:], zrow[:], axis=mybir.AxisListType.X)
                nc.gpsimd.dma_start(out[b:b + 1, :], spart[:],
                                    accum_op=mybir.AluOpType.add)



    nc.compile()
    return nc


_compiled = None


def _get_compiled():
    global _compiled
    if _compiled is None:
        _compiled = build_bass()
    return _compiled


def kernel(x, adj, clinical, W1, b1, W2, b2, lw1, lb1, lw2, lb2):
    x = np.ascontiguousarray(np.asarray(x, dtype=np.float32))
    adj = np.asarray(adj, dtype=np.float32)
    clinical = np.ascontiguousarray(np.asarray(clinical, dtype=np.float32))
    W1 = np.ascontiguousarray(np.asarray(W1, dtype=np.float32))
    b1 = np.ascontiguousarray(np.asarray(b1, dtype=np.float32))
    W2 = np.ascontiguousarray(np.asarray(W2, dtype=np.float32))
    b2 = np.ascontiguousarray(np.asarray(b2, dtype=np.float32))
    lw1 = np.ascontiguousarray(np.asarray(lw1, dtype=np.float32))
    lb1 = np.ascontiguousarray(np.asarray(lb1, dtype=np.float32))
    lw2 = np.ascontiguousarray(np.asarray(lw2, dtype=np.float32))
    lb2 = np.ascontiguousarray(np.asarray(lb2, dtype=np.float32))

    nc = _get_compiled()

    xT = np.ascontiguousarray(x.transpose(0, 2, 1))   # [B, F, PP]
    adjT = np.ascontiguousarray(adj.T)                # [PP, PP]

    in_maps = []
    for core in range(NCORES):
        sl = slice(core * BPC, (core + 1) * BPC)
        in_maps.append({
            "xT": xT[sl], "adjT": adjT, "clin": clinical[sl],
            "W1": W1, "b1": b1, "W2": W2, "b2": b2,
            "lw1": lw1, "lb1": lb1, "lw2": lw2, "lb2": lb2,
        })

    res = bass_utils.run_bass_kernel_spmd(nc, in_maps, core_ids=list(range(NCORES)))
    return np.concatenate([res.results[c]["out"] for c in range(NCORES)], axis=0)
